# revision 11
# baseline (speedup 1.0000x reference)
"""Trainium2 Bass kernel for masked multi-head attention.

Problem: B=4, S=2048, D=768, H=12 (head_dim=64), boolean prune mask per
head, softmax over keys, out-projection.

Sharding (8 cores): data-parallel over batch (4) x tensor-parallel over
head halves (2 x 6 heads).  Core c handles batch c//2 and heads
(c%2)*6 .. (c%2)*6+5.  Host sums the two partial out-projections per
batch and adds out_b.

Design:
  * QKV projections and the out-projection run in bf16.  q/k biases are
    folded into the PSUM->fp8 cast via ACT Identity-with-bias; the v
    bias (+ per-head ones column for softmax denominators) is a K=1
    bf16 ones-row matmul.
  * Scores fold the prune mask INTO an fp8 DoubleRow matmul: stationary
    planes (k-features, -240*I), moving planes (q-features, 1-mask).
    -240 is exactly representable in TRN e4m3 and shifts masked scores
    far enough negative that both exp paths round them to (-)0.
  * exp is split across TWO engines per (kt,qh) tile: ACT native exp
    (fp8 out, scale=1/8) and DVE single-op Schraudolph: the fp32->int8
    convert of score*log2e + (7-sigma)*8 IS the e4m3 bitpattern of
    exp(score/8); saturation to -128 = e4m3 -0.0 handles the mask
    sentinel.  Split tuned so ACT/DVE both stay under the PE's per-head
    matmul time -> heads run PE-bound.
  * ctx DoubleRow matmuls contract 256 key positions per step; odd
    heads live in partitions 64-127 end-to-end.
  * Normalization: denominator rows via ones columns, recip emitted
    early (kt==2 of the next head) so the K=1 broadcast matmuls never
    stall the PE at kt==8; recb cast on Pool.
  * Tail: head 5's normalization is chunked by 512-column blocks, each
    chunk feeding its 4 out-projection tiles immediately; ot copies
    alternate DVE/ACT and output DMAs alternate sync/gpsimd queues.
"""

import os
import sys
import math

import numpy as np

try:
    import concourse.bass as bass
except ImportError:  # pragma: no cover - path fallback for fresh dirs
    for _p in ("/opt/trn_rl_repo", "/root/.axon_site/_ro/trn_rl_repo"):
        if os.path.isdir(_p) and _p not in sys.path:
            sys.path.insert(0, _p)
    import concourse.bass as bass

import ml_dtypes
import concourse.mybir as mybir
from concourse import bacc
from concourse.tile import TileContext
from concourse.bass_utils import run_bass_kernel_spmd

E4 = ml_dtypes.float8_e4m3
BF16 = ml_dtypes.bfloat16
F8 = mybir.dt.float8e4
F32 = mybir.dt.float32
I8 = mybir.dt.int8
BBF = mybir.dt.bfloat16
DR = mybir.MatmulPerfMode.DoubleRow

B, S, D, H = 4, 2048, 768, 12
HD = 64          # head dim
HPC = 6          # heads per core
FPC = HPC * HD   # features per core (384)
VW = HPC * (HD + 1)  # 390
NCORES = 8
KT = S // 128    # 16 key tiles
ST = S // 128    # 16 seq tiles

# Schraudolph int8 fast exp: int8(s*C8 + D8) bitcast e4m3 ~= exp(s/8).
# fp32->int8 saturates (-128 = e4m3 -0.0) and rounds to nearest even.
C8 = math.log2(math.e)
D8 = (7.0 - 0.05792) * 8.0

# DVE-exp tile count per head (of 32); rest go to ACT.
DVE_N = (8, 12, 12, 13, 13, 13)


def _dve_tiles(n):
    # Bresenham spread of n DVE tiles over the 32 (kt, qh) slots
    return frozenset(i for i in range(32)
                     if (i + 1) * n // 32 > i * n // 32)


_CACHE = {}
_last_result = None


def _build_bass():
    nc = bacc.Bacc()

    hsT = nc.declare_dram_parameter("hsT", [D, S], BBF, isOutput=False)
    wqT = nc.declare_dram_parameter("wqT", [D, FPC], BBF, isOutput=False)
    wkT = nc.declare_dram_parameter("wkT", [D, FPC], BBF, isOutput=False)
    wvT = nc.declare_dram_parameter("wvT", [D, VW], BBF, isOutput=False)
    wvb = nc.declare_dram_parameter("wvb", [1, VW], BBF, isOutput=False)
    qkb = nc.declare_dram_parameter("qkb", [128, 6], F32, isOutput=False)
    owT = nc.declare_dram_parameter("owT", [3, 128, D], BBF, isOutput=False)
    dgI = nc.declare_dram_parameter("dgI", [128, S], F8, isOutput=False)
    zrow = nc.declare_dram_parameter("zrow", [64, S], F8, isOutput=False)
    maskP = nc.declare_dram_parameter("maskP", [HPC, KT, 128, S], F8,
                                      isOutput=False)
    out = nc.declare_dram_parameter("out", [S, D], BBF, isOutput=True)

    EXP = mybir.ActivationFunctionType.Exp
    IDENT = mybir.ActivationFunctionType.Identity
    COPYF = mybir.ActivationFunctionType.Copy
    MULT = mybir.AluOpType.mult
    ADD = mybir.AluOpType.add

    with TileContext(nc) as tc, \
            tc.tile_pool(name="persist", bufs=1) as pp, \
            tc.tile_pool(name="qmp", bufs=4) as qm_pool, \
            tc.tile_pool(name="pbuf", bufs=3) as p_pool, \
            tc.tile_pool(name="obuf", bufs=3) as o_pool, \
            tc.tile_pool(name="pswork", bufs=2, space="PSUM") as ps_pool, \
            tc.tile_pool(name="psctx", bufs=1, space="PSUM") as ctx_pool:

        # ---------------- persistent SBUF tensors + input DMAs ----------
        # Queue plan (per-engine FIFO = emission order; transfers stripe
        # across all 16 DMA engines, so only trigger cost serializes):
        #   sync:   wq, hsT[0:1024] c0 c3, wk, nb23 c0 c3, dgI/zrow
        #           h2 h3 h5 -> mask kt-even, ow (emitted before head 5)
        #   scalar: hsT[0:1024] c1 c4, qkb, wvb, dgI/zrow h0 -> qk casts
        #   gpsimd: head-0 kt0/kt2 masks, hsT[0:1024] c2 c5, wv,
        #           nb23 c2 c5, dgI/zrow h1 h4 -> mask kt-odd, qm copies,
        #           recb, sums
        hsT_sb = [pp.tile([128, S], BBF, name=f"hsT{c}", tag=f"hsT{c}")
                  for c in range(6)]
        kS = [pp.tile([128, 2, S], F8, name=f"kS{h}", tag=f"kS{h}")
              for h in range(HPC)]
        qT8 = [pp.tile([128, S], F8, name=f"qT8{t}", tag=f"qT8{t}")
               for t in range(3)]
        v6P = [pp.tile([128, 2, HPC * 128], F8, name=f"v6P{i}", tag=f"v6P{i}")
               for i in range(8)]
        qm_bufs = [qm_pool.tile([128, 2, S], F8, tag="qm", name=f"qm0_{kt}")
                   for kt in range(4)]

        ones_sb = pp.tile([1, 512], BBF, name="ones_sb", tag="ones_sb")
        nc.vector.memset(ones_sb, 1.0)
        warm_sb = pp.tile([128, 512], BBF, name="warm_sb", tag="warm_sb")
        nc.vector.memset(warm_sb, 0.0)
        # head-0 kt0/kt2 masks first so they land before the first scores
        nc.gpsimd.dma_start(out=qm_bufs[0][:, 1, :], in_=maskP[0, 0])
        nc.gpsimd.dma_start(out=qm_bufs[2][:, 1, :], in_=maskP[0, 2])

        def load_w(handle, width, nm, eng):
            tiles = [pp.tile([128, width], BBF, name=f"{nm}{c}",
                             tag=f"{nm}{c}") for c in range(6)]
            for c in range(6):
                eng.dma_start(out=tiles[c],
                              in_=handle[c * 128:(c + 1) * 128, :])
            return tiles

        wq_sb = load_w(wqT, FPC, "wq", nc.sync)
        for c in range(6):
            eng = (nc.sync, nc.scalar, nc.gpsimd)[c % 3]
            eng.dma_start(out=hsT_sb[c][:, 0:1024],
                          in_=hsT[c * 128:(c + 1) * 128, 0:1024])
        wk_sb = load_w(wkT, FPC, "wk", nc.sync)
        qkb_sb = pp.tile([128, 6], F32, name="qkb_sb", tag="qkb_sb")
        nc.scalar.dma_start(out=qkb_sb, in_=qkb[:, :])
        wvb_sb = pp.tile([1, VW], BBF, name="wvb_sb", tag="wvb_sb")
        nc.scalar.dma_start(out=wvb_sb, in_=wvb[:, :])
        wv_sb = load_w(wvT, VW, "wv", nc.gpsimd)
        for c in range(6):
            eng = (nc.sync, nc.scalar, nc.gpsimd)[c % 3]
            eng.dma_start(out=hsT_sb[c][:, 1024:2048],
                          in_=hsT[c * 128:(c + 1) * 128, 1024:2048])

        # per-head score stationaries [128, 2, S]: plane0 = k-features
        # (rows (h%2)*64..+63; other rows DMA'd zero), plane1 = -240*I
        for h in range(HPC):
            eng = (nc.scalar, nc.gpsimd, nc.sync, nc.sync,
                   nc.gpsimd, nc.sync)[h]
            eng.dma_start(out=kS[h][:, 1, :], in_=dgI[:, :])
            r = slice(64, 128) if h % 2 == 0 else slice(0, 64)
            eng.dma_start(out=kS[h][r, 0, :], in_=zrow[:, :])

        ow_sb = [pp.tile([128, D], BBF, name=f"ow{c}", tag=f"ow{c}")
                 for c in range(3)]

        # v6P unused-column zero fills (DVE; cheap strided fp8 writes)
        def v6p_memset(i):
            v4 = v6P[i].rearrange("p t (h c) -> p t h c", c=128)
            nc.vector.memset(v4[:, :, 0::2, HD + 1:], 0.0)
            nc.vector.memset(v4[:, :, 1::2, 0:HD - 1], 0.0)

        for i in range(8):
            v6p_memset(i)

        # PE warm-up while input DMAs land (HAM clock gate)
        warm_ps = ps_pool.tile([128, 512], F32, tag="work", name="warm_ps")
        for _ in range(14):
            nc.tensor.matmul(warm_ps, lhsT=warm_sb[:, 0:128], rhs=warm_sb,
                             start=True, stop=True)
        nc.vector.tensor_copy(out=warm_sb[:, 0:1], in_=warm_ps[:, 0:1])
        # trigger the exp table load early (off the critical path)
        exp_pre = pp.tile([1, 16], F32, name="exp_pre", tag="exp_pre")
        nc.scalar.activation(exp_pre, warm_ps[0:1, 0:16], EXP)

        ctxu = [pp.tile([128, S], BBF, name=f"ctxu{h}", tag=f"ctxu{h}")
                for h in range(HPC)]
        ctxa = [pp.tile([128, S], BBF, name=f"ctxa{t}", tag=f"ctxa{t}")
                for t in range(3)]
        # single-row norm scratch: one buffer, reused head to head ([1,S]
        # tiles still reserve their columns on every partition)
        sums1 = [pp.tile([1, S], F32, name=f"sums1_{h}", tag="sums1",
                         bufs=2) for h in range(HPC)]
        recf1 = [pp.tile([1, S], F32, name=f"recf1_{h}", tag="recf1",
                         bufs=2) for h in range(HPC)]
        recb1 = [pp.tile([1, S], BBF, name=f"recb1_{h}", tag="recb1",
                         bufs=2) for h in range(HPC)]

        # ---------------- projection emitters ----------------------------
        # prework psums use the "work" slots: tiles interleaved inside
        # head 0 must never wait on the ctx slot (deadlock via v_tile ->
        # ctx dependency).
        def qk_chunk(w_sb, is_q, t, nb):
            ps = ps_pool.tile([128, 512], F32, tag="work",
                              name=f"qk{int(is_q)}_{t}_{nb}")
            for c in range(6):
                nc.tensor.matmul(
                    ps,
                    lhsT=w_sb[c][:, t * 128:(t + 1) * 128],
                    rhs=hsT_sb[c][:, nb * 512:(nb + 1) * 512],
                    start=(c == 0), stop=(c == 5))
            ns = slice(nb * 512, (nb + 1) * 512)
            # psum->fp8 cast with fused bias on ACT (Identity w/ bias AP)
            if is_q:
                nc.scalar.activation(qT8[t][:, ns], ps, IDENT,
                                     bias=qkb_sb[:, t:t + 1])
            else:
                nc.scalar.activation(kS[2 * t][0:64, 0, ns], ps[0:64],
                                     IDENT, bias=qkb_sb[0:64, 3 + t:4 + t])
                nc.scalar.activation(kS[2 * t + 1][64:128, 0, ns],
                                     ps[64:128], IDENT,
                                     bias=qkb_sb[64:128, 3 + t:4 + t])

        def v_tile(t):
            ps = ps_pool.tile([128, VW], F32, tag="work", name=f"vps{t}")
            for c in range(6):
                nc.tensor.matmul(
                    ps,
                    lhsT=hsT_sb[c][:, t * 128:(t + 1) * 128],
                    rhs=wv_sb[c],
                    start=(c == 0), stop=False)
            nc.tensor.matmul(ps, lhsT=ones_sb[:, 0:128], rhs=wvb_sb,
                             start=False, stop=True)
            dst = v6P[t // 2][:, t % 2, :].rearrange("p (h c) -> p h c", c=128)
            src = ps.rearrange("p (h c) -> p h c", c=HD + 1)
            nc.vector.tensor_copy(out=dst[:, 0::2, 0:HD + 1], in_=src[:, 0::2])
            nc.vector.tensor_copy(out=dst[:, 1::2, HD - 1:128], in_=src[:, 1::2])

        # ---------------- attention, head by head -----------------------
        # prework is interleaved into head 0 so PE slack absorbs it while
        # ACT/DVE stream exp.
        for nb in range(2):
            qk_chunk(wq_sb, True, 0, nb)
            qk_chunk(wk_sb, False, 0, nb)
        v_tile(0)
        v_tile(1)
        for nb in range(2, 4):
            qk_chunk(wq_sb, True, 0, nb)
            qk_chunk(wk_sb, False, 0, nb)

        def head(h, qm_bufs):
            dve_set = _dve_tiles(DVE_N[h])
            ctx_ps = ctx_pool.tile([128, S], F32, tag="ctx", name=f"ctx{h}")
            pP_cur = [None]
            for kt in range(KT):
                if h % 2 == 0 and kt < 4:
                    if h == 0:
                        qm = qm_bufs[kt]
                    else:
                        qm = qm_pool.tile([128, 2, S], F8, tag="qm",
                                          name=f"qm{h}_{kt}")
                        qm_bufs[kt] = qm
                    nc.gpsimd.tensor_copy(out=qm[:, 0, :], in_=qT8[h // 2])
                elif h % 2 == 0:
                    qm = qm_bufs[kt % 4]
                else:
                    # stagger so the first kts reuse bufs freed earliest
                    # by the previous head
                    qm = qm_bufs[(kt + 2) % 4]
                if not (h == 0 and kt in (0, 2)):
                    dma_eng = nc.sync if kt % 2 == 0 else nc.gpsimd
                    dma_eng.dma_start(out=qm[:, 1, :], in_=maskP[h, kt])
                if kt % 2 == 0:
                    pP_cur[0] = p_pool.tile([128, 2, S], F8, tag="p",
                                            name=f"p{h}_{kt}")
                pP = pP_cur[0]
                sts = []
                for qh in range(2):
                    st = ps_pool.tile([128, 1024], F32, tag="work",
                                      name=f"st{h}_{kt}_{qh}")
                    for i in range(2):
                        q0 = qh * 1024 + i * 512
                        nc.tensor.matmul(
                            st[:, i * 512:(i + 1) * 512],
                            lhsT=kS[h][:, :, kt * 128:(kt + 1) * 128],
                            rhs=qm[:, :, q0:q0 + 512],
                            start=True, stop=True, perf_mode=DR)
                    sts.append(st)
                for qh in range(2):
                    dst = pP[:, kt % 2, qh * 1024:(qh + 1) * 1024]
                    if (kt * 2 + qh) in dve_set:
                        nc.vector.tensor_scalar(dst.bitcast(I8), sts[qh],
                                                C8, D8, MULT, ADD)
                    else:
                        nc.scalar.activation(dst, sts[qh], EXP,
                                             scale=1.0 / math.sqrt(HD))
                # fine-grained prework interleave: one group per kt so
                # the PE never owes a long block while ACT starves
                if h == 0 and 0 <= kt < 14:
                    v_tile(kt + 2)
                elif h == 1 and kt < 8:
                    qk_chunk(wq_sb if kt % 2 == 0 else wk_sb,
                             kt % 2 == 0, 1, kt // 2)
                elif h == 2 and kt < 8:
                    qk_chunk(wq_sb if kt % 2 == 0 else wk_sb,
                             kt % 2 == 0, 2, kt // 2)
                if kt == 2 and h > 0:
                    norm_recip(h - 1)
                if kt == 8 and h > 0:
                    norm_apply(h - 1)
                if kt % 2 == 1:
                    i = kt // 2
                    for j in range(4):
                        q0 = j * 512
                        nc.tensor.matmul(
                            ctx_ps[:, q0:q0 + 512],
                            lhsT=v6P[i][:, :, h * 128:(h + 1) * 128],
                            rhs=pP[:, :, q0:q0 + 512],
                            start=(i == 0), stop=(i == 7), perf_mode=DR)
            # head epilogue: ctx rows + denominator row to SBUF split by
            # column halves across DVE and ACT, then the f32 sums row via
            # a casting SWDGE DMA (no engine cost).
            if h % 2 == 0:
                nc.vector.tensor_copy(out=ctxu[h][0:HD + 1, 0:1024],
                                      in_=ctx_ps[0:HD + 1, 0:1024])
                nc.scalar.activation(ctxu[h][0:HD + 1, 1024:2048],
                                     ctx_ps[0:HD + 1, 1024:2048], COPYF)
                nc.gpsimd.dma_start(out=sums1[h], in_=ctxu[h][HD:HD + 1, :])
            else:
                # partition-base rule: <=32 partitions from base 32, 64
                # from base 64 -> two copies (row 63 = denominator)
                nc.vector.tensor_copy(out=ctxu[h][32:64, 0:1024],
                                      in_=ctx_ps[32:64, 0:1024])
                nc.vector.tensor_copy(out=ctxu[h][64:128, 0:1024],
                                      in_=ctx_ps[64:128, 0:1024])
                nc.scalar.activation(ctxu[h][32:64, 1024:2048],
                                     ctx_ps[32:64, 1024:2048], COPYF)
                nc.scalar.activation(ctxu[h][64:128, 1024:2048],
                                     ctx_ps[64:128, 1024:2048], COPYF)
                if h < HPC - 1:
                    nc.gpsimd.dma_start(out=sums1[h],
                                        in_=ctxu[h][HD - 1:HD, :])
                else:
                    # tail: split so chunk-0 recip starts off the DVE half
                    nc.gpsimd.dma_start(out=sums1[h][:, 0:1024],
                                        in_=ctxu[h][HD - 1:HD, 0:1024])
                    nc.gpsimd.dma_start(out=sums1[h][:, 1024:2048],
                                        in_=ctxu[h][HD - 1:HD, 1024:2048])

        def norm_recip(h):
            nc.vector.reciprocal_approx_fast(out=recf1[h], in_=sums1[h])
            nc.gpsimd.tensor_copy(out=recb1[h], in_=recf1[h])

        def norm_apply(h):
            # normalization for head h: broadcast 1/sums across the
            # head's 64-row band via a K=1 ones outer-product matmul.
            ro = (h % 2) * 64
            for qh in range(2):
                rb = ps_pool.tile([128, 1024], F32, tag="work",
                                  name=f"rb{h}_{qh}")
                for i in range(2):
                    q0 = qh * 1024 + i * 512
                    nc.tensor.matmul(
                        rb[ro:ro + 64, i * 512:(i + 1) * 512],
                        lhsT=ones_sb[:, 0:64],
                        rhs=recb1[h][:, q0:q0 + 512],
                        start=True, stop=True)
                nc.vector.tensor_tensor(
                    ctxa[h // 2][ro:ro + 64, qh * 1024:(qh + 1) * 1024],
                    ctxu[h][ro:ro + 64, qh * 1024:(qh + 1) * 1024],
                    rb[ro:ro + 64, :], MULT)

        for h in range(HPC):
            if h == HPC - 1:
                # out-projection weights: issued here so the sync engine
                # loads them during head 5, ready for the tail
                for c in range(3):
                    nc.sync.dma_start(out=ow_sb[c], in_=owT[c])
            head(h, qm_bufs)

        # ---------------- tail: chunked head-5 norm + out projection ----
        # keep the PE array busy through the first norm chunk's latency
        warm2_ps = ps_pool.tile([128, 512], F32, tag="work", name="warm2_ps")
        for _ in range(12):
            nc.tensor.matmul(warm2_ps, lhsT=warm_sb[:, 0:128], rhs=warm_sb,
                             start=True, stop=True)

        hL = HPC - 1
        for ck in range(4):
            cs = slice(ck * 512, (ck + 1) * 512)
            nc.vector.reciprocal_approx_fast(out=recf1[hL][:, cs],
                                             in_=sums1[hL][:, cs])
            nc.gpsimd.tensor_copy(out=recb1[hL][:, cs], in_=recf1[hL][:, cs])
            rb = ctx_pool.tile([128, 512], F32, tag="ctx", name=f"rbt{ck}")
            nc.tensor.matmul(rb[64:128, :], lhsT=ones_sb[:, 0:64],
                             rhs=recb1[hL][:, cs], start=True, stop=True)
            nc.vector.tensor_tensor(ctxa[2][64:128, cs],
                                    ctxu[hL][64:128, cs],
                                    rb[64:128, :], MULT)
            # out projection for the 4 seq tiles this chunk covers
            for si in range(ck * 4, ck * 4 + 4):
                op = ps_pool.tile([128, 1024], F32, tag="work", name=f"o{si}")
                for c in range(3):
                    nc.tensor.matmul(
                        op[:, 0:512],
                        lhsT=ctxa[c][:, si * 128:(si + 1) * 128],
                        rhs=ow_sb[c][:, 0:512],
                        start=(c == 0), stop=(c == 2))
                    nc.tensor.matmul(
                        op[:, 512:D],
                        lhsT=ctxa[c][:, si * 128:(si + 1) * 128],
                        rhs=ow_sb[c][:, 512:D],
                        start=(c == 0), stop=(c == 2))
                ot = o_pool.tile([128, D], BBF, tag="o", name=f"ot{si}")
                if si % 2 == 0:
                    nc.vector.tensor_copy(out=ot, in_=op[:, 0:D])
                else:
                    nc.scalar.copy(out=ot, in_=op[:, 0:D])
                dma_eng = nc.sync if si % 2 == 0 else nc.gpsimd
                dma_eng.dma_start(out=out[si * 128:(si + 1) * 128, :], in_=ot)

    return nc


def _get_nc(finalized=False):
    if "nc" not in _CACHE:
        _CACHE["nc"] = _build_bass()
    nc = _CACHE["nc"]
    if finalized and not nc.is_finalized():
        nc.finalize()
    return nc


def _prep_core_inputs(inputs, core):
    """Host-side shard prep for one core."""
    hs = np.asarray(inputs["hidden_states"], np.float32)
    mask = np.asarray(inputs["attention_mask"])
    q_w = np.asarray(inputs["q_w"], np.float32)
    q_b = np.asarray(inputs["q_b"], np.float32)
    k_w = np.asarray(inputs["k_w"], np.float32)
    k_b = np.asarray(inputs["k_b"], np.float32)
    v_w = np.asarray(inputs["v_w"], np.float32)
    v_b = np.asarray(inputs["v_b"], np.float32)
    out_w = np.asarray(inputs["out_w"], np.float32)

    b, hh = divmod(core, 2)
    hsl = slice(hh * FPC, (hh + 1) * FPC)

    # per-head 65-wide v blocks; even heads (v0..v63, ones) -> ctx rows
    # 0-63 + denom row 64; odd heads (ones, v0..v63) so the on-chip
    # scatter to cols 63..127 puts ones/denom at row 63, v at 64-127.
    wv65 = np.zeros((D, VW), np.float32)
    wvbv = np.zeros((1, VW), np.float32)
    for j in range(HPC):
        fs = hh * FPC + j * HD
        off = j * (HD + 1) + (j % 2)
        wv65[:, off:off + HD] = v_w[fs:fs + HD].T
        wvbv[0, off:off + HD] = v_b[fs:fs + HD]
        wvbv[0, j * (HD + 1) + (0 if j % 2 else HD)] = 1.0

    qkbv = np.empty((128, 6), np.float32)
    qkbv[:, 0:3] = q_b[hsl].reshape(3, 128).T
    qkbv[:, 3:6] = k_b[hsl].reshape(3, 128).T

    dgIv = np.tile(-240.0 * np.eye(128, dtype=np.float32), (1, KT)).astype(E4)

    heads = slice(hh * HPC, (hh + 1) * HPC)
    mT6 = np.ascontiguousarray(mask[0, heads].transpose(0, 2, 1))
    # (1 - m) as fp8 bytes: 1.0 -> 0x38, 0.0 -> 0x00
    mP = np.where(mT6, np.uint8(0), np.uint8(0x38)).reshape(HPC, KT, 128, S)

    return {
        "hsT": np.ascontiguousarray(hs[b].T).astype(BF16),
        "wqT": np.ascontiguousarray(q_w[hsl].T).astype(BF16),
        "wkT": np.ascontiguousarray(k_w[hsl].T).astype(BF16),
        "wvT": wv65.astype(BF16),
        "wvb": wvbv.astype(BF16), "qkb": qkbv,
        "owT": np.ascontiguousarray(
            out_w.T[hsl].reshape(3, 128, D)).astype(BF16),
        "dgI": dgIv, "zrow": np.zeros((64, S), E4), "maskP": mP.view(E4),
    }


def kernel(**inputs):
    global _last_result
    nc = _get_nc(finalized=True)
    in_maps = [_prep_core_inputs(inputs, c) for c in range(NCORES)]
    res = run_bass_kernel_spmd(
        nc, in_maps, core_ids=list(range(NCORES)),
        tmpdir=os.environ.get("KERNEL_TRACE_DIR") or None)
    _last_result = res
    outs = [np.asarray(r["out"], dtype=np.float32) for r in res.results]
    out_b = np.asarray(inputs["out_b"], np.float32)
    full = np.empty((B, S, D), np.float32)
    for b in range(B):
        full[b] = outs[2 * b] + outs[2 * b + 1] + out_b
    return full


# revision 14
# speedup vs baseline: 1.1586x; 1.1586x over previous
"""Trainium2 Bass kernel for masked multi-head attention.

Problem: B=4, S=2048, D=768, H=12 (head_dim=64), boolean prune mask per
head, softmax over keys, out-projection.

Sharding (8 cores): data-parallel over batch (4) x tensor-parallel over
head halves (2 x 6 heads).  Core c handles batch c//2 and heads
(c%2)*6 .. (c%2)*6+5.  Host sums the two partial out-projections per
batch and adds out_b.

Design:
  * QKV projections and the out-projection run in bf16.  q/k biases are
    folded into the PSUM->fp8 cast via ACT Identity-with-bias; the v
    bias (+ per-head ones column for softmax denominators) is a K=1
    bf16 ones-row matmul.
  * Scores fold the prune mask INTO an fp8 DoubleRow matmul: stationary
    planes (k-features, -240*I), moving planes (q-features, 1-mask).
    -240 is exactly representable in TRN e4m3 and shifts masked scores
    far enough negative that both exp paths round them to (-)0.
  * exp is split across TWO engines per (kt,qh) tile: ACT native exp
    (fp8 out, scale=1/8) and DVE single-op Schraudolph: the fp32->int8
    convert of score*log2e + (7-sigma)*8 IS the e4m3 bitpattern of
    exp(score/8); saturation to -128 = e4m3 -0.0 handles the mask
    sentinel.  Split tuned so ACT/DVE both stay under the PE's per-head
    matmul time -> heads run PE-bound.
  * ctx DoubleRow matmuls contract 256 key positions per step; odd
    heads live in partitions 64-127 end-to-end.
  * Normalization: denominator rows via ones columns, recip emitted
    early (kt==2 of the next head) so the K=1 broadcast matmuls never
    stall the PE at kt==8; recb cast on Pool.
  * Tail: head 5's normalization is chunked by 512-column blocks, each
    chunk feeding its 4 out-projection tiles immediately; ot copies
    alternate DVE/ACT and output DMAs alternate sync/gpsimd queues.
"""

import os
import sys
import math

import numpy as np

try:
    import concourse.bass as bass
except ImportError:  # pragma: no cover - path fallback for fresh dirs
    for _p in ("/opt/trn_rl_repo", "/root/.axon_site/_ro/trn_rl_repo"):
        if os.path.isdir(_p) and _p not in sys.path:
            sys.path.insert(0, _p)
    import concourse.bass as bass

import ml_dtypes
import concourse.mybir as mybir
from concourse import bacc
from concourse.tile import TileContext
from concourse.bass_utils import run_bass_kernel_spmd

E4 = ml_dtypes.float8_e4m3
BF16 = ml_dtypes.bfloat16
F8 = mybir.dt.float8e4
F32 = mybir.dt.float32
I8 = mybir.dt.int8
BBF = mybir.dt.bfloat16
DR = mybir.MatmulPerfMode.DoubleRow

B, S, D, H = 4, 2048, 768, 12
HD = 64          # head dim
HPC = 6          # heads per core
FPC = HPC * HD   # features per core (384)
VW = HPC * (HD + 1)  # 390
NCORES = 8
KT = S // 128    # 16 key tiles
ST = S // 128    # 16 seq tiles

# Schraudolph int8 fast exp: int8(s*C8 + D8) bitcast e4m3 ~= exp(s/8).
# fp32->int8 saturates (-128 = e4m3 -0.0) and rounds to nearest even.
C8 = math.log2(math.e)
D8 = (7.0 - 0.05792) * 8.0

# DVE-exp tile count per head (of 32); rest go to ACT.
DVE_N = (8, 12, 12, 13, 13, 13)


def _dve_tiles(n):
    # Bresenham spread of n DVE tiles over the 32 (kt, qh) slots
    return frozenset(i for i in range(32)
                     if (i + 1) * n // 32 > i * n // 32)


_CACHE = {}
_last_result = None


def _build_bass():
    nc = bacc.Bacc()

    hsT = nc.declare_dram_parameter("hsT", [D, S], BBF, isOutput=False)
    wqT = nc.declare_dram_parameter("wqT", [D, FPC], BBF, isOutput=False)
    wkT = nc.declare_dram_parameter("wkT", [D, FPC], BBF, isOutput=False)
    wvT = nc.declare_dram_parameter("wvT", [D, VW], BBF, isOutput=False)
    wvb = nc.declare_dram_parameter("wvb", [1, VW], BBF, isOutput=False)
    qkb = nc.declare_dram_parameter("qkb", [128, 6], F32, isOutput=False)
    owT = nc.declare_dram_parameter("owT", [3, 128, D], BBF, isOutput=False)
    dgI = nc.declare_dram_parameter("dgI", [128, S], F8, isOutput=False)
    zrow = nc.declare_dram_parameter("zrow", [64, S], F8, isOutput=False)
    maskP = nc.declare_dram_parameter("maskP", [HPC, KT, 128, S], F8,
                                      isOutput=False)
    out = nc.declare_dram_parameter("out", [S, D], BBF, isOutput=True)

    EXP = mybir.ActivationFunctionType.Exp
    IDENT = mybir.ActivationFunctionType.Identity
    COPYF = mybir.ActivationFunctionType.Copy
    MULT = mybir.AluOpType.mult
    ADD = mybir.AluOpType.add

    with TileContext(nc) as tc, \
            tc.tile_pool(name="persist", bufs=1) as pp, \
            tc.tile_pool(name="qmp", bufs=4) as qm_pool, \
            tc.tile_pool(name="pbuf", bufs=3) as p_pool, \
            tc.tile_pool(name="obuf", bufs=3) as o_pool, \
            tc.tile_pool(name="pswork", bufs=2, space="PSUM") as ps_pool, \
            tc.tile_pool(name="psctx", bufs=1, space="PSUM") as ctx_pool:

        # ---------------- persistent SBUF tensors + input DMAs ----------
        # Queue plan (per-engine FIFO = emission order; transfers stripe
        # across all 16 DMA engines, so only trigger cost serializes):
        #   sync:   wq, hsT[0:1024] c0 c3, wk, nb23 c0 c3, dgI/zrow
        #           h2 h3 h5 -> mask kt-even, ow (emitted before head 5)
        #   scalar: hsT[0:1024] c1 c4, qkb, wvb, dgI/zrow h0 -> qk casts
        #   gpsimd: head-0 kt0/kt2 masks, hsT[0:1024] c2 c5, wv,
        #           nb23 c2 c5, dgI/zrow h1 h4 -> mask kt-odd, qm copies,
        #           recb, sums
        hsT_sb = [pp.tile([128, S], BBF, name=f"hsT{c}", tag=f"hsT{c}")
                  for c in range(6)]
        kS = [pp.tile([128, 2, S], F8, name=f"kS{h}", tag=f"kS{h}")
              for h in range(HPC)]
        qT8 = [pp.tile([128, S], F8, name=f"qT8{t}", tag=f"qT8{t}")
               for t in range(3)]
        v6P = [pp.tile([128, 2, HPC * 128], F8, name=f"v6P{i}", tag=f"v6P{i}")
               for i in range(8)]
        qm_bufs = [qm_pool.tile([128, 2, S], F8, tag="qm", name=f"qm0_{kt}")
                   for kt in range(4)]

        ones_sb = pp.tile([1, 512], BBF, name="ones_sb", tag="ones_sb")
        nc.vector.memset(ones_sb, 1.0)
        warm_sb = pp.tile([128, 512], BBF, name="warm_sb", tag="warm_sb")
        nc.vector.memset(warm_sb, 0.0)
        # head-0 kt0/kt2 masks first so they land before the first scores
        nc.gpsimd.dma_start(out=qm_bufs[0][:, 1, :], in_=maskP[0, 0])
        nc.gpsimd.dma_start(out=qm_bufs[2][:, 1, :], in_=maskP[0, 2])

        def load_w(handle, width, nm, eng):
            tiles = [pp.tile([128, width], BBF, name=f"{nm}{c}",
                             tag=f"{nm}{c}") for c in range(6)]
            for c in range(6):
                eng.dma_start(out=tiles[c],
                              in_=handle[c * 128:(c + 1) * 128, :])
            return tiles

        wq_sb = load_w(wqT, FPC, "wq", nc.sync)
        for c in range(6):
            eng = (nc.sync, nc.scalar, nc.gpsimd)[c % 3]
            eng.dma_start(out=hsT_sb[c][:, 0:1024],
                          in_=hsT[c * 128:(c + 1) * 128, 0:1024])
        wk_sb = load_w(wkT, FPC, "wk", nc.sync)
        qkb_sb = pp.tile([128, 6], F32, name="qkb_sb", tag="qkb_sb")
        nc.scalar.dma_start(out=qkb_sb, in_=qkb[:, :])
        wvb_sb = pp.tile([1, VW], BBF, name="wvb_sb", tag="wvb_sb")
        nc.scalar.dma_start(out=wvb_sb, in_=wvb[:, :])
        wv_sb = load_w(wvT, VW, "wv", nc.gpsimd)
        for c in range(6):
            eng = (nc.sync, nc.scalar, nc.gpsimd)[c % 3]
            eng.dma_start(out=hsT_sb[c][:, 1024:2048],
                          in_=hsT[c * 128:(c + 1) * 128, 1024:2048])

        # per-head score stationaries [128, 2, S]: plane0 = k-features
        # (rows (h%2)*64..+63; other rows DMA'd zero), plane1 = -240*I
        for h in range(HPC):
            eng = (nc.scalar, nc.gpsimd, nc.sync, nc.sync,
                   nc.gpsimd, nc.sync)[h]
            eng.dma_start(out=kS[h][:, 1, :], in_=dgI[:, :])
            r = slice(64, 128) if h % 2 == 0 else slice(0, 64)
            eng.dma_start(out=kS[h][r, 0, :], in_=zrow[:, :])

        ow_sb = [pp.tile([128, D], BBF, name=f"ow{c}", tag=f"ow{c}")
                 for c in range(3)]

        # v6P unused-column zero fills (DVE; cheap strided fp8 writes)
        def v6p_memset(i):
            v4 = v6P[i].rearrange("p t (h c) -> p t h c", c=128)
            nc.vector.memset(v4[:, :, 0::2, HD + 1:], 0.0)
            nc.vector.memset(v4[:, :, 1::2, 0:HD - 1], 0.0)

        for i in range(8):
            v6p_memset(i)

        # PE warm-up while input DMAs land (HAM clock gate)
        warm_ps = ps_pool.tile([128, 512], F32, tag="work", name="warm_ps")
        for _ in range(14):
            nc.tensor.matmul(warm_ps, lhsT=warm_sb[:, 0:128], rhs=warm_sb,
                             start=True, stop=True)
        nc.vector.tensor_copy(out=warm_sb[:, 0:1], in_=warm_ps[:, 0:1])
        # trigger the exp table load early (off the critical path)
        exp_pre = pp.tile([1, 16], F32, name="exp_pre", tag="exp_pre")
        nc.scalar.activation(exp_pre, warm_ps[0:1, 0:16], EXP)

        ctxu = [pp.tile([128, S], BBF, name=f"ctxu{h}", tag=f"ctxu{h}")
                for h in range(HPC)]
        ctxa = [pp.tile([128, S], BBF, name=f"ctxa{t}", tag=f"ctxa{t}")
                for t in range(3)]
        # single-row norm scratch: one buffer, reused head to head ([1,S]
        # tiles still reserve their columns on every partition)
        sums1 = [pp.tile([1, S], F32, name=f"sums1_{h}", tag="sums1",
                         bufs=2) for h in range(HPC)]
        recf1 = [pp.tile([1, S], F32, name=f"recf1_{h}", tag="recf1",
                         bufs=2) for h in range(HPC)]
        recb1 = [pp.tile([1, S], BBF, name=f"recb1_{h}", tag="recb1",
                         bufs=2) for h in range(HPC)]

        # ---------------- projection emitters ----------------------------
        # prework psums use the "work" slots: tiles interleaved inside
        # head 0 must never wait on the ctx slot (deadlock via v_tile ->
        # ctx dependency).
        def qk_chunk(w_sb, is_q, t, nb):
            ps = ps_pool.tile([128, 512], F32, tag="work",
                              name=f"qk{int(is_q)}_{t}_{nb}")
            for c in range(6):
                nc.tensor.matmul(
                    ps,
                    lhsT=w_sb[c][:, t * 128:(t + 1) * 128],
                    rhs=hsT_sb[c][:, nb * 512:(nb + 1) * 512],
                    start=(c == 0), stop=(c == 5))
            ns = slice(nb * 512, (nb + 1) * 512)
            # psum->fp8 cast with fused bias on ACT (Identity w/ bias AP)
            if is_q:
                nc.scalar.activation(qT8[t][:, ns], ps, IDENT,
                                     bias=qkb_sb[:, t:t + 1])
            else:
                nc.scalar.activation(kS[2 * t][0:64, 0, ns], ps[0:64],
                                     IDENT, bias=qkb_sb[0:64, 3 + t:4 + t])
                nc.scalar.activation(kS[2 * t + 1][64:128, 0, ns],
                                     ps[64:128], IDENT,
                                     bias=qkb_sb[64:128, 3 + t:4 + t])

        def v_tile(t):
            ps = ps_pool.tile([128, VW], F32, tag="work", name=f"vps{t}")
            for c in range(6):
                nc.tensor.matmul(
                    ps,
                    lhsT=hsT_sb[c][:, t * 128:(t + 1) * 128],
                    rhs=wv_sb[c],
                    start=(c == 0), stop=False)
            nc.tensor.matmul(ps, lhsT=ones_sb[:, 0:128], rhs=wvb_sb,
                             start=False, stop=True)
            dst = v6P[t // 2][:, t % 2, :].rearrange("p (h c) -> p h c", c=128)
            src = ps.rearrange("p (h c) -> p h c", c=HD + 1)
            nc.vector.tensor_copy(out=dst[:, 0::2, 0:HD + 1], in_=src[:, 0::2])
            nc.vector.tensor_copy(out=dst[:, 1::2, HD - 1:128], in_=src[:, 1::2])

        # ---------------- attention, head by head -----------------------
        # prework is interleaved into head 0 so PE slack absorbs it while
        # ACT/DVE stream exp.
        for nb in range(2):
            qk_chunk(wq_sb, True, 0, nb)
            qk_chunk(wk_sb, False, 0, nb)
        v_tile(0)
        v_tile(1)
        for nb in range(2, 4):
            qk_chunk(wq_sb, True, 0, nb)
            qk_chunk(wk_sb, False, 0, nb)

        def head(h, qm_bufs):
            dve_set = _dve_tiles(DVE_N[h])
            ctx_ps = ctx_pool.tile([128, S], F32, tag="ctx", name=f"ctx{h}")
            pP_cur = [None]
            for kt in range(KT):
                if h % 2 == 0 and kt < 4:
                    if h == 0:
                        qm = qm_bufs[kt]
                    else:
                        qm = qm_pool.tile([128, 2, S], F8, tag="qm",
                                          name=f"qm{h}_{kt}")
                        qm_bufs[kt] = qm
                    # SBUF->SBUF DMA: only the trigger costs engine time
                    nc.gpsimd.dma_start(out=qm[:, 0, :], in_=qT8[h // 2])
                elif h % 2 == 0:
                    qm = qm_bufs[kt % 4]
                else:
                    # stagger so the first kts reuse bufs freed earliest
                    # by the previous head
                    qm = qm_bufs[(kt + 2) % 4]
                if not (h == 0 and kt in (0, 2)):
                    dma_eng = nc.sync if kt % 2 == 0 else nc.gpsimd
                    dma_eng.dma_start(out=qm[:, 1, :], in_=maskP[h, kt])
                if kt % 2 == 0:
                    pP_cur[0] = p_pool.tile([128, 2, S], F8, tag="p",
                                            name=f"p{h}_{kt}")
                pP = pP_cur[0]
                sts = []
                for qh in range(2):
                    st = ps_pool.tile([128, 1024], F32, tag="work",
                                      name=f"st{h}_{kt}_{qh}")
                    for i in range(2):
                        q0 = qh * 1024 + i * 512
                        nc.tensor.matmul(
                            st[:, i * 512:(i + 1) * 512],
                            lhsT=kS[h][:, :, kt * 128:(kt + 1) * 128],
                            rhs=qm[:, :, q0:q0 + 512],
                            start=True, stop=True, perf_mode=DR)
                    sts.append(st)
                for qh in range(2):
                    dst = pP[:, kt % 2, qh * 1024:(qh + 1) * 1024]
                    if (kt * 2 + qh) in dve_set:
                        nc.vector.tensor_scalar(dst.bitcast(I8), sts[qh],
                                                C8, D8, MULT, ADD)
                    else:
                        nc.scalar.activation(dst, sts[qh], EXP,
                                             scale=1.0 / math.sqrt(HD))
                # fine-grained prework interleave: one group per kt so
                # the PE never owes a long block while ACT starves
                if h == 0 and 0 <= kt < 14:
                    v_tile(kt + 2)
                elif h == 1 and kt < 8:
                    qk_chunk(wq_sb if kt % 2 == 0 else wk_sb,
                             kt % 2 == 0, 1, kt // 2)
                elif h == 2 and kt < 8:
                    qk_chunk(wq_sb if kt % 2 == 0 else wk_sb,
                             kt % 2 == 0, 2, kt // 2)
                if kt == 2 and h > 0:
                    norm_recip(h - 1)
                if kt == 8 and h > 0:
                    norm_apply(h - 1)
                if kt % 2 == 1:
                    i = kt // 2
                    for j in range(4):
                        q0 = j * 512
                        nc.tensor.matmul(
                            ctx_ps[:, q0:q0 + 512],
                            lhsT=v6P[i][:, :, h * 128:(h + 1) * 128],
                            rhs=pP[:, :, q0:q0 + 512],
                            start=(i == 0), stop=(i == 7), perf_mode=DR)
            # head epilogue: ctx rows + denominator row to SBUF split by
            # column halves across DVE and ACT, then the f32 sums row via
            # a casting SWDGE DMA (no engine cost).
            if h % 2 == 0:
                nc.vector.tensor_copy(out=ctxu[h][0:HD + 1, 0:1024],
                                      in_=ctx_ps[0:HD + 1, 0:1024])
                nc.scalar.activation(ctxu[h][0:HD + 1, 1024:2048],
                                     ctx_ps[0:HD + 1, 1024:2048], COPYF)
                nc.gpsimd.dma_start(out=sums1[h], in_=ctxu[h][HD:HD + 1, :])
            else:
                # partition-base rule: <=32 partitions from base 32, 64
                # from base 64 -> two copies (row 63 = denominator)
                nc.vector.tensor_copy(out=ctxu[h][32:64, 0:1024],
                                      in_=ctx_ps[32:64, 0:1024])
                nc.vector.tensor_copy(out=ctxu[h][64:128, 0:1024],
                                      in_=ctx_ps[64:128, 0:1024])
                nc.scalar.activation(ctxu[h][32:64, 1024:2048],
                                     ctx_ps[32:64, 1024:2048], COPYF)
                nc.scalar.activation(ctxu[h][64:128, 1024:2048],
                                     ctx_ps[64:128, 1024:2048], COPYF)
                if h < HPC - 1:
                    nc.gpsimd.dma_start(out=sums1[h],
                                        in_=ctxu[h][HD - 1:HD, :])
                else:
                    # tail: split so chunk-0 recip starts off the DVE half
                    nc.gpsimd.dma_start(out=sums1[h][:, 0:1024],
                                        in_=ctxu[h][HD - 1:HD, 0:1024])
                    nc.gpsimd.dma_start(out=sums1[h][:, 1024:2048],
                                        in_=ctxu[h][HD - 1:HD, 1024:2048])

        def norm_recip(h):
            nc.vector.reciprocal_approx_fast(out=recf1[h], in_=sums1[h])
            # casting SBUF->SBUF DMA (f32 -> bf16), off-engine
            nc.gpsimd.dma_start(out=recb1[h], in_=recf1[h])

        def norm_apply(h):
            # normalization for head h: broadcast 1/sums across the
            # head's 64-row band via a K=1 ones outer-product matmul.
            ro = (h % 2) * 64
            for qh in range(2):
                rb = ps_pool.tile([128, 1024], F32, tag="work",
                                  name=f"rb{h}_{qh}")
                for i in range(2):
                    q0 = qh * 1024 + i * 512
                    nc.tensor.matmul(
                        rb[ro:ro + 64, i * 512:(i + 1) * 512],
                        lhsT=ones_sb[:, 0:64],
                        rhs=recb1[h][:, q0:q0 + 512],
                        start=True, stop=True)
                nc.vector.tensor_tensor(
                    ctxa[h // 2][ro:ro + 64, qh * 1024:(qh + 1) * 1024],
                    ctxu[h][ro:ro + 64, qh * 1024:(qh + 1) * 1024],
                    rb[ro:ro + 64, :], MULT)

        for h in range(HPC):
            if h == HPC - 1:
                # out-projection weights: issued here so the sync engine
                # loads them during head 5, ready for the tail
                for c in range(3):
                    nc.sync.dma_start(out=ow_sb[c], in_=owT[c])
            head(h, qm_bufs)

        # ---------------- tail: chunked head-5 norm + out projection ----
        # keep the PE array busy through the first norm chunk's latency
        warm2_ps = ps_pool.tile([128, 512], F32, tag="work", name="warm2_ps")
        for _ in range(12):
            nc.tensor.matmul(warm2_ps, lhsT=warm_sb[:, 0:128], rhs=warm_sb,
                             start=True, stop=True)

        hL = HPC - 1
        for ck in range(4):
            cs = slice(ck * 512, (ck + 1) * 512)
            nc.vector.reciprocal_approx_fast(out=recf1[hL][:, cs],
                                             in_=sums1[hL][:, cs])
            nc.gpsimd.dma_start(out=recb1[hL][:, cs], in_=recf1[hL][:, cs])
            rb = ctx_pool.tile([128, 512], F32, tag="ctx", name=f"rbt{ck}")
            nc.tensor.matmul(rb[64:128, :], lhsT=ones_sb[:, 0:64],
                             rhs=recb1[hL][:, cs], start=True, stop=True)
            nc.vector.tensor_tensor(ctxa[2][64:128, cs],
                                    ctxu[hL][64:128, cs],
                                    rb[64:128, :], MULT)
            # out projection for the 4 seq tiles this chunk covers
            for si in range(ck * 4, ck * 4 + 4):
                op = ps_pool.tile([128, 1024], F32, tag="work", name=f"o{si}")
                for c in range(3):
                    nc.tensor.matmul(
                        op[:, 0:512],
                        lhsT=ctxa[c][:, si * 128:(si + 1) * 128],
                        rhs=ow_sb[c][:, 0:512],
                        start=(c == 0), stop=(c == 2))
                    nc.tensor.matmul(
                        op[:, 512:D],
                        lhsT=ctxa[c][:, si * 128:(si + 1) * 128],
                        rhs=ow_sb[c][:, 512:D],
                        start=(c == 0), stop=(c == 2))
                ot = o_pool.tile([128, D], BBF, tag="o", name=f"ot{si}")
                if si % 2 == 0:
                    nc.vector.tensor_copy(out=ot, in_=op[:, 0:D])
                else:
                    nc.scalar.copy(out=ot, in_=op[:, 0:D])
                dma_eng = nc.sync if si % 2 == 0 else nc.gpsimd
                dma_eng.dma_start(out=out[si * 128:(si + 1) * 128, :], in_=ot)

    return nc


def _get_nc(finalized=False):
    if "nc" not in _CACHE:
        _CACHE["nc"] = _build_bass()
    nc = _CACHE["nc"]
    if finalized and not nc.is_finalized():
        nc.finalize()
    return nc


def _prep_core_inputs(inputs, core):
    """Host-side shard prep for one core."""
    hs = np.asarray(inputs["hidden_states"], np.float32)
    mask = np.asarray(inputs["attention_mask"])
    q_w = np.asarray(inputs["q_w"], np.float32)
    q_b = np.asarray(inputs["q_b"], np.float32)
    k_w = np.asarray(inputs["k_w"], np.float32)
    k_b = np.asarray(inputs["k_b"], np.float32)
    v_w = np.asarray(inputs["v_w"], np.float32)
    v_b = np.asarray(inputs["v_b"], np.float32)
    out_w = np.asarray(inputs["out_w"], np.float32)

    b, hh = divmod(core, 2)
    hsl = slice(hh * FPC, (hh + 1) * FPC)

    # per-head 65-wide v blocks; even heads (v0..v63, ones) -> ctx rows
    # 0-63 + denom row 64; odd heads (ones, v0..v63) so the on-chip
    # scatter to cols 63..127 puts ones/denom at row 63, v at 64-127.
    wv65 = np.zeros((D, VW), np.float32)
    wvbv = np.zeros((1, VW), np.float32)
    for j in range(HPC):
        fs = hh * FPC + j * HD
        off = j * (HD + 1) + (j % 2)
        wv65[:, off:off + HD] = v_w[fs:fs + HD].T
        wvbv[0, off:off + HD] = v_b[fs:fs + HD]
        wvbv[0, j * (HD + 1) + (0 if j % 2 else HD)] = 1.0

    qkbv = np.empty((128, 6), np.float32)
    qkbv[:, 0:3] = q_b[hsl].reshape(3, 128).T
    qkbv[:, 3:6] = k_b[hsl].reshape(3, 128).T

    dgIv = np.tile(-240.0 * np.eye(128, dtype=np.float32), (1, KT)).astype(E4)

    heads = slice(hh * HPC, (hh + 1) * HPC)
    mT6 = np.ascontiguousarray(mask[0, heads].transpose(0, 2, 1))
    # (1 - m) as fp8 bytes: 1.0 -> 0x38, 0.0 -> 0x00
    mP = np.where(mT6, np.uint8(0), np.uint8(0x38)).reshape(HPC, KT, 128, S)

    return {
        "hsT": np.ascontiguousarray(hs[b].T).astype(BF16),
        "wqT": np.ascontiguousarray(q_w[hsl].T).astype(BF16),
        "wkT": np.ascontiguousarray(k_w[hsl].T).astype(BF16),
        "wvT": wv65.astype(BF16),
        "wvb": wvbv.astype(BF16), "qkb": qkbv,
        "owT": np.ascontiguousarray(
            out_w.T[hsl].reshape(3, 128, D)).astype(BF16),
        "dgI": dgIv, "zrow": np.zeros((64, S), E4), "maskP": mP.view(E4),
    }


def kernel(**inputs):
    global _last_result
    nc = _get_nc(finalized=True)
    in_maps = [_prep_core_inputs(inputs, c) for c in range(NCORES)]
    res = run_bass_kernel_spmd(
        nc, in_maps, core_ids=list(range(NCORES)),
        tmpdir=os.environ.get("KERNEL_TRACE_DIR") or None)
    _last_result = res
    outs = [np.asarray(r["out"], dtype=np.float32) for r in res.results]
    out_b = np.asarray(inputs["out_b"], np.float32)
    full = np.empty((B, S, D), np.float32)
    for b in range(B):
        full[b] = outs[2 * b] + outs[2 * b + 1] + out_b
    return full


# revision 42
# speedup vs baseline: 1.1847x; 1.0226x over previous
"""Trainium2 Bass kernel for masked multi-head attention.

Problem: B=4, S=2048, D=768, H=12 (head_dim=64), boolean prune mask per
head, softmax over keys, out-projection.

Sharding (8 cores): data-parallel over batch (4) x tensor-parallel over
head halves (2 x 6 heads).  Core c handles batch c//2 and heads
(c%2)*6 .. (c%2)*6+5.  Host sums the two partial out-projections per
batch and adds out_b.

Design:
  * QKV projections and the out-projection run in bf16.  q/k biases are
    folded into the PSUM->fp8 cast via ACT Identity-with-bias; the v
    bias (+ per-head ones column for softmax denominators) is a K=1
    bf16 ones-row matmul.
  * Scores fold the prune mask INTO an fp8 DoubleRow matmul: stationary
    planes (k-features, -240*I), moving planes (q-features, 1-mask).
    -240 is exactly representable in TRN e4m3 and shifts masked scores
    far enough negative that both exp paths round them to (-)0.
  * exp is split across TWO engines per (kt,qh) tile: ACT native exp
    (fp8 out, scale=1/8) and DVE single-op Schraudolph: the fp32->int8
    convert of score*log2e + (7-sigma)*8 IS the e4m3 bitpattern of
    exp(score/8); saturation to -128 = e4m3 -0.0 handles the mask
    sentinel.  Split tuned so ACT/DVE both stay under the PE's per-head
    matmul time -> heads run PE-bound.
  * ctx DoubleRow matmuls contract 256 key positions per step; odd
    heads live in partitions 64-127 end-to-end.
  * Normalization: denominator rows via ones columns, recip emitted
    early (kt==2 of the next head) so the K=1 broadcast matmuls never
    stall the PE at kt==8; recb cast on Pool.
  * Tail: head 5's normalization is chunked by 512-column blocks, each
    chunk feeding its 4 out-projection tiles immediately; ot copies
    alternate DVE/ACT and output DMAs alternate sync/gpsimd queues.
"""

import os
import sys
import math

import numpy as np

try:
    import concourse.bass as bass
except ImportError:  # pragma: no cover - path fallback for fresh dirs
    for _p in ("/opt/trn_rl_repo", "/root/.axon_site/_ro/trn_rl_repo"):
        if os.path.isdir(_p) and _p not in sys.path:
            sys.path.insert(0, _p)
    import concourse.bass as bass

import ml_dtypes
import concourse.mybir as mybir
from concourse import bacc
from concourse.tile import TileContext
from concourse.bass_utils import run_bass_kernel_spmd

E4 = ml_dtypes.float8_e4m3
BF16 = ml_dtypes.bfloat16
F8 = mybir.dt.float8e4
F32 = mybir.dt.float32
I8 = mybir.dt.int8
BBF = mybir.dt.bfloat16
DR = mybir.MatmulPerfMode.DoubleRow

B, S, D, H = 4, 2048, 768, 12
HD = 64          # head dim
HPC = 6          # heads per core
FPC = HPC * HD   # features per core (384)
VW = HPC * (HD + 1)  # 390
NCORES = 8
KT = S // 128    # 16 key tiles
ST = S // 128    # 16 seq tiles

# Schraudolph int8 fast exp: int8(s*C8 + D8) bitcast e4m3 ~= exp(s/8).
# fp32->int8 saturates (-128 = e4m3 -0.0) and rounds to nearest even.
C8 = math.log2(math.e)
D8 = (7.0 - 0.05792) * 8.0

# DVE-exp tile count per head (of 32); rest go to ACT.
DVE_N = (8, 12, 12, 14, 14, 14)


def _dve_tiles(n):
    # Bresenham spread of n DVE tiles over the 32 (kt, qh) slots
    return frozenset(i for i in range(32)
                     if (i + 1) * n // 32 > i * n // 32)


_CACHE = {}
_last_result = None


def _build_bass():
    nc = bacc.Bacc()

    hsT = nc.declare_dram_parameter("hsT", [D, S], BBF, isOutput=False)
    wqT = nc.declare_dram_parameter("wqT", [D, FPC], BBF, isOutput=False)
    wkT = nc.declare_dram_parameter("wkT", [D, FPC], BBF, isOutput=False)
    wvT = nc.declare_dram_parameter("wvT", [D, VW], BBF, isOutput=False)
    wvb = nc.declare_dram_parameter("wvb", [1, VW], BBF, isOutput=False)
    qkb = nc.declare_dram_parameter("qkb", [128, 6], F32, isOutput=False)
    owT = nc.declare_dram_parameter("owT", [3, 128, D], BBF, isOutput=False)
    dgI = nc.declare_dram_parameter("dgI", [128, S], F8, isOutput=False)
    zrow = nc.declare_dram_parameter("zrow", [64, S], F8, isOutput=False)
    maskP = nc.declare_dram_parameter("maskP", [HPC, KT, 128, S], F8,
                                      isOutput=False)
    out = nc.declare_dram_parameter("out", [S, D], BBF, isOutput=True)

    EXP = mybir.ActivationFunctionType.Exp
    IDENT = mybir.ActivationFunctionType.Identity
    COPYF = mybir.ActivationFunctionType.Copy
    MULT = mybir.AluOpType.mult
    ADD = mybir.AluOpType.add

    with TileContext(nc) as tc, \
            tc.tile_pool(name="persist", bufs=1) as pp, \
            tc.tile_pool(name="qmp", bufs=4) as qm_pool, \
            tc.tile_pool(name="pbuf", bufs=3) as p_pool, \
            tc.tile_pool(name="obuf", bufs=3) as o_pool, \
            tc.tile_pool(name="pswork", bufs=2, space="PSUM") as ps_pool, \
            tc.tile_pool(name="psctx", bufs=1, space="PSUM") as ctx_pool:

        # ---------------- persistent SBUF tensors + input DMAs ----------
        # Queue plan (per-engine FIFO = emission order; transfers stripe
        # across all 16 DMA engines, so only trigger cost serializes).
        # The Pool engine starts ~13us late and SWDGE adds multi-us
        # latency, so nothing needed before ~20us rides on gpsimd.
        #   sync:   mask(0,0), wq, hsT[0:1024] c even, wk,
        #           hsT[1024:] c even, dgI/zrow h2 h3 h5
        #           -> mask kt-even, sums, ow (before head 5), out tiles
        #   scalar: mask(0,2), hsT[0:1024] c odd, mask(0,1), mask(0,3),
        #           qkb, wvb, wv, dgI/zrow h0 -> qk casts
        #   gpsimd: hsT[1024:] c odd, dgI/zrow h1 h4
        #           -> mask kt-odd (>=kt5), qm q-plane DMAs, recb DMAs
        hsT_sb = [pp.tile([128, S], BBF, name=f"hsT{c}", tag=f"hsT{c}")
                  for c in range(6)]
        kS = [pp.tile([128, 2, S], F8, name=f"kS{h}", tag=f"kS{h}")
              for h in range(HPC)]
        qT8 = [pp.tile([128, S], F8, name=f"qT8{t}", tag=f"qT8{t}")
               for t in range(3)]
        v6P = [pp.tile([128, 2, HPC * 128], F8, name=f"v6P{i}", tag=f"v6P{i}")
               for i in range(8)]
        qm_bufs = [qm_pool.tile([128, 2, S], F8, tag="qm", name=f"qm0_{kt}")
                   for kt in range(4)]

        ones_sb = pp.tile([1, 512], BBF, name="ones_sb", tag="ones_sb")
        nc.vector.memset(ones_sb, 1.0)
        warm_sb = pp.tile([128, 512], BBF, name="warm_sb", tag="warm_sb")
        nc.vector.memset(warm_sb, 0.0)
        # head-0 masks first so they land before the first scores
        nc.sync.dma_start(out=qm_bufs[0][:, 1, :], in_=maskP[0, 0])
        nc.scalar.dma_start(out=qm_bufs[2][:, 1, :], in_=maskP[0, 2])

        def load_w(handle, width, nm, eng):
            tiles = [pp.tile([128, width], BBF, name=f"{nm}{c}",
                             tag=f"{nm}{c}") for c in range(6)]
            for c in range(6):
                eng.dma_start(out=tiles[c],
                              in_=handle[c * 128:(c + 1) * 128, :])
            return tiles

        wq_sb = load_w(wqT, FPC, "wq", nc.sync)
        for c in range(6):
            eng = nc.sync if c % 2 == 0 else nc.scalar
            eng.dma_start(out=hsT_sb[c][:, 0:1024],
                          in_=hsT[c * 128:(c + 1) * 128, 0:1024])
        nc.scalar.dma_start(out=qm_bufs[1][:, 1, :], in_=maskP[0, 1])
        nc.scalar.dma_start(out=qm_bufs[3][:, 1, :], in_=maskP[0, 3])
        wk_sb = load_w(wkT, FPC, "wk", nc.sync)
        qkb_sb = pp.tile([128, 6], F32, name="qkb_sb", tag="qkb_sb")
        nc.scalar.dma_start(out=qkb_sb, in_=qkb[:, :])
        wvb_sb = pp.tile([1, VW], BBF, name="wvb_sb", tag="wvb_sb")
        nc.scalar.dma_start(out=wvb_sb, in_=wvb[:, :])
        wv_sb = load_w(wvT, VW, "wv", nc.scalar)
        for c in range(6):
            eng = nc.sync if c % 2 == 0 else nc.gpsimd
            eng.dma_start(out=hsT_sb[c][:, 1024:2048],
                          in_=hsT[c * 128:(c + 1) * 128, 1024:2048])

        # per-head score stationaries [128, 2, S]: plane0 = k-features
        # (rows (h%2)*64..+63; other rows DMA'd zero), plane1 = -240*I
        for h in range(HPC):
            eng = (nc.scalar, nc.gpsimd, nc.sync, nc.sync,
                   nc.gpsimd, nc.sync)[h]
            eng.dma_start(out=kS[h][:, 1, :], in_=dgI[:, :])
            r = slice(64, 128) if h % 2 == 0 else slice(0, 64)
            eng.dma_start(out=kS[h][r, 0, :], in_=zrow[:, :])

        ow_sb = [pp.tile([128, D], BBF, name=f"ow{c}", tag=f"ow{c}")
                 for c in range(3)]

        # v6P unused-column zero fills (DVE; cheap strided fp8 writes)
        def v6p_memset(i):
            v4 = v6P[i].rearrange("p t (h c) -> p t h c", c=128)
            nc.vector.memset(v4[:, :, 0::2, HD + 1:], 0.0)
            nc.vector.memset(v4[:, :, 1::2, 0:HD - 1], 0.0)

        for i in range(8):
            v6p_memset(i)

        # PE warm-up while input DMAs land (HAM clock gate)
        warm_ps = ps_pool.tile([128, 512], F32, tag="work", name="warm_ps")
        for _ in range(14):
            nc.tensor.matmul(warm_ps, lhsT=warm_sb[:, 0:128], rhs=warm_sb,
                             start=True, stop=True)
        nc.vector.tensor_copy(out=warm_sb[:, 0:1], in_=warm_ps[:, 0:1])
        # trigger the exp table load early (off the critical path)
        exp_pre = pp.tile([1, 16], F32, name="exp_pre", tag="exp_pre")
        nc.scalar.activation(exp_pre, warm_ps[0:1, 0:16], EXP)

        ctxu = [pp.tile([128, S], BBF, name=f"ctxu{h}", tag=f"ctxu{h}")
                for h in range(HPC)]
        ctxa = [pp.tile([128, S], BBF, name=f"ctxa{t}", tag=f"ctxa{t}")
                for t in range(3)]
        # norm scratch.  Mid-head sums land as [4, 512] (DMA reshape) so
        # the DVE reciprocal runs at 4-partition width; the tail keeps a
        # [65, S] tiles whose row 64 carries the last head's chain.
        sums4 = {h: pp.tile([4, 512], F32, name=f"sums4_{h}", tag="sums4",
                            bufs=2) for h in (0, 1, 2, 3, 5)}
        recf4 = {h: pp.tile([4, 512], F32, name=f"recf4_{h}", tag="recf4",
                            bufs=2) for h in (0, 1, 2, 3, 5)}
        recb1 = {h: pp.tile([1, S], BBF, name=f"recb1_{h}", tag="recb1",
                            bufs=2) for h in (0, 1, 2, 3, 5)}
        # tail (head 4, processed last): denominator row lives on
        # partition 64 end-to-end, so the norm chain needs no DMA.
        # reciprocal_approx_fast (custom DVE op) rejects base-64 size-1
        # and column-sliced APs, so copy/recip run per 512-col chunk on
        # full-width [128, 512] tiles over the [64:128) band (rows
        # 65..127 of head 4's ctx psum are zeros; their recips unused).
        sumsTc = [pp.tile([128, 512], F32, name=f"sumsTc{ck}", tag="sumsTc",
                          bufs=4) for ck in range(4)]
        recfTc = [pp.tile([128, 512], F32, name=f"recfTc{ck}", tag="recfTc",
                          bufs=4) for ck in range(4)]
        recbT = pp.tile([65, S], BBF, name="recbT", tag="recbT")
        ones64 = pp.tile([65, 64], BBF, name="ones64", tag="ones64")
        nc.vector.memset(ones64[64:65, :], 1.0)

        # ---------------- projection emitters ----------------------------
        # prework psums use the "work" slots: tiles interleaved inside
        # head 0 must never wait on the ctx slot (deadlock via v_tile ->
        # ctx dependency).
        def qk_chunk(w_sb, is_q, t, nb):
            ps = ps_pool.tile([128, 512], F32, tag="work",
                              name=f"qk{int(is_q)}_{t}_{nb}")
            for c in range(6):
                nc.tensor.matmul(
                    ps,
                    lhsT=w_sb[c][:, t * 128:(t + 1) * 128],
                    rhs=hsT_sb[c][:, nb * 512:(nb + 1) * 512],
                    start=(c == 0), stop=(c == 5))
            ns = slice(nb * 512, (nb + 1) * 512)
            # psum->fp8 cast with fused bias on ACT (Identity w/ bias AP)
            if is_q:
                nc.scalar.activation(qT8[t][:, ns], ps, IDENT,
                                     bias=qkb_sb[:, t:t + 1])
            else:
                nc.scalar.activation(kS[2 * t][0:64, 0, ns], ps[0:64],
                                     IDENT, bias=qkb_sb[0:64, 3 + t:4 + t])
                nc.scalar.activation(kS[2 * t + 1][64:128, 0, ns],
                                     ps[64:128], IDENT,
                                     bias=qkb_sb[64:128, 3 + t:4 + t])

        def v_tile(t):
            ps = ps_pool.tile([128, VW], F32, tag="work", name=f"vps{t}")
            for c in range(6):
                nc.tensor.matmul(
                    ps,
                    lhsT=hsT_sb[c][:, t * 128:(t + 1) * 128],
                    rhs=wv_sb[c],
                    start=(c == 0), stop=False)
            nc.tensor.matmul(ps, lhsT=ones_sb[:, 0:128], rhs=wvb_sb,
                             start=False, stop=True)
            dst = v6P[t // 2][:, t % 2, :].rearrange("p (h c) -> p h c", c=128)
            src = ps.rearrange("p (h c) -> p h c", c=HD + 1)
            nc.vector.tensor_copy(out=dst[:, 0::2, 0:HD + 1], in_=src[:, 0::2])
            nc.vector.tensor_copy(out=dst[:, 1::2, HD - 1:128], in_=src[:, 1::2])

        # ---------------- attention, head by head -----------------------
        # prework is interleaved into head 0 so PE slack absorbs it while
        # ACT/DVE stream exp.
        for nb in range(2):
            qk_chunk(wq_sb, True, 0, nb)
            qk_chunk(wk_sb, False, 0, nb)
        v_tile(0)
        v_tile(1)
        for nb in range(2, 4):
            qk_chunk(wq_sb, True, 0, nb)
            qk_chunk(wk_sb, False, 0, nb)

        def head(h, pos, first_of_pair, prev_h, qm_bufs):
            dve_set = _dve_tiles(DVE_N[pos])
            ctx_ps = ctx_pool.tile([128, S], F32, tag="ctx", name=f"ctx{h}")
            pP_cur = [None]
            for kt in range(KT):
                if first_of_pair and kt < 4:
                    if h == 0:
                        qm = qm_bufs[kt]
                    else:
                        qm = qm_pool.tile([128, 2, S], F8, tag="qm",
                                          name=f"qm{h}_{kt}")
                        qm_bufs[kt] = qm
                    # SBUF->SBUF DMA: only the trigger costs engine time
                    nc.gpsimd.dma_start(out=qm[:, 0, :], in_=qT8[h // 2])
                elif first_of_pair:
                    qm = qm_bufs[kt % 4]
                else:
                    # stagger so the first kts reuse bufs freed earliest
                    # by the previous head
                    qm = qm_bufs[(kt + 2) % 4]
                if not (h == 0 and kt < 4):
                    dma_eng = nc.sync if kt % 2 == 0 else nc.gpsimd
                    dma_eng.dma_start(out=qm[:, 1, :], in_=maskP[h, kt])
                if kt % 2 == 0:
                    pP_cur[0] = p_pool.tile([128, 2, S], F8, tag="p",
                                            name=f"p{h}_{kt}")
                pP = pP_cur[0]
                sts = []
                for qh in range(2):
                    st = ps_pool.tile([128, 1024], F32, tag="work",
                                      name=f"st{h}_{kt}_{qh}")
                    for i in range(2):
                        q0 = qh * 1024 + i * 512
                        nc.tensor.matmul(
                            st[:, i * 512:(i + 1) * 512],
                            lhsT=kS[h][:, :, kt * 128:(kt + 1) * 128],
                            rhs=qm[:, :, q0:q0 + 512],
                            start=True, stop=True, perf_mode=DR)
                    sts.append(st)
                for qh in range(2):
                    dst = pP[:, kt % 2, qh * 1024:(qh + 1) * 1024]
                    if (kt * 2 + qh) in dve_set:
                        nc.vector.tensor_scalar(dst.bitcast(I8), sts[qh],
                                                C8, D8, MULT, ADD)
                    else:
                        nc.scalar.activation(dst, sts[qh], EXP,
                                             scale=1.0 / math.sqrt(HD))
                # fine-grained prework interleave: one group per kt so
                # the PE never owes a long block while ACT starves
                if h == 0 and 0 <= kt < 14:
                    v_tile(kt + 2)
                elif h == 1 and kt < 8:
                    qk_chunk(wq_sb if kt % 2 == 0 else wk_sb,
                             kt % 2 == 0, 1, kt // 2)
                elif h == 2 and kt < 8:
                    qk_chunk(wq_sb if kt % 2 == 0 else wk_sb,
                             kt % 2 == 0, 2, kt // 2)
                if kt == 2 and prev_h is not None:
                    norm_recip(prev_h)
                if kt == 8 and prev_h is not None:
                    norm_apply(prev_h)
                if kt % 2 == 1:
                    i = kt // 2
                    for j in range(4):
                        q0 = j * 512
                        nc.tensor.matmul(
                            ctx_ps[:, q0:q0 + 512],
                            lhsT=v6P[i][:, :, h * 128:(h + 1) * 128],
                            rhs=pP[:, :, q0:q0 + 512],
                            start=(i == 0), stop=(i == 7), perf_mode=DR)
            # head epilogue: ctx rows + denominator row to SBUF split by
            # column halves across DVE and ACT, then the f32 sums row via
            # a casting SWDGE DMA (no engine cost).  Head 4 is processed
            # LAST: its chain is handled by the chunked tail instead.
            if h == 4:
                return ctx_ps
            if h % 2 == 0:
                nc.vector.tensor_copy(out=ctxu[h][0:HD + 1, 0:1024],
                                      in_=ctx_ps[0:HD + 1, 0:1024])
                nc.scalar.activation(ctxu[h][0:HD + 1, 1024:2048],
                                     ctx_ps[0:HD + 1, 1024:2048], COPYF)
                nc.gpsimd.dma_start(out=sums4[h], in_=ctxu[h][HD:HD + 1, :])
            else:
                # partition-base rule: <=32 partitions from base 32, 64
                # from base 64 -> two copies (row 63 = denominator)
                nc.vector.tensor_copy(out=ctxu[h][32:64, 0:1024],
                                      in_=ctx_ps[32:64, 0:1024])
                nc.vector.tensor_copy(out=ctxu[h][64:128, 0:1024],
                                      in_=ctx_ps[64:128, 0:1024])
                nc.scalar.activation(ctxu[h][32:64, 1024:2048],
                                     ctx_ps[32:64, 1024:2048], COPYF)
                nc.scalar.activation(ctxu[h][64:128, 1024:2048],
                                     ctx_ps[64:128, 1024:2048], COPYF)
                nc.gpsimd.dma_start(out=sums4[h],
                                    in_=ctxu[h][HD - 1:HD, :])
            return ctx_ps

        def norm_recip(h):
            nc.vector.reciprocal_approx_fast(out=recf4[h], in_=sums4[h])
            # casting + reshaping SBUF->SBUF DMA ([4,512] f32 -> [1,S]
            # bf16); SWDGE latency is fine here (needed 6 kts later)
            nc.gpsimd.dma_start(out=recb1[h], in_=recf4[h])

        def norm_apply(h):
            # normalization for head h: broadcast 1/sums across the
            # head's 64-row band via a K=1 ones outer-product matmul.
            ro = (h % 2) * 64
            for qh in range(2):
                rb = ps_pool.tile([128, 1024], F32, tag="work",
                                  name=f"rb{h}_{qh}")
                for i in range(2):
                    q0 = qh * 1024 + i * 512
                    nc.tensor.matmul(
                        rb[ro:ro + 64, i * 512:(i + 1) * 512],
                        lhsT=ones_sb[:, 0:64],
                        rhs=recb1[h][:, q0:q0 + 512],
                        start=True, stop=True)
                nc.vector.tensor_tensor(
                    ctxa[h // 2][ro:ro + 64, qh * 1024:(qh + 1) * 1024],
                    ctxu[h][ro:ro + 64, qh * 1024:(qh + 1) * 1024],
                    rb[ro:ro + 64, :], MULT)

        # head 4 runs LAST: even slot puts its denominator on partition
        # 64 (32-aligned), so the tail norm chain stays on DVE (no DMA)
        ORDER = (0, 1, 2, 3, 5, 4)
        ctxL = None
        for pos, h in enumerate(ORDER):
            if h == 5:
                # out-projection weights: issued here so the sync engine
                # loads them during heads 5/4, ready for the tail
                for c in range(3):
                    nc.sync.dma_start(out=ow_sb[c], in_=owT[c])
            prev_h = ORDER[pos - 1] if pos > 0 else None
            ctxL = head(h, pos, h in (0, 2, 5), prev_h, qm_bufs)

        # ---------------- tail: chunked head-4 norm + out projection ----
        # keep the PE array busy through the first norm chunk's latency
        warm2_ps = ps_pool.tile([128, 512], F32, tag="work", name="warm2_ps")
        for _ in range(12):
            nc.tensor.matmul(warm2_ps, lhsT=warm_sb[:, 0:128], rhs=warm_sb,
                             start=True, stop=True)

        hL = 4
        for ck in range(4):
            cs = slice(ck * 512, (ck + 1) * 512)
            # ctx rows for this chunk on ACT, denominator chain on DVE;
            # all ops sit on the partitions they already use
            nc.scalar.activation(ctxu[hL][0:HD, cs], ctxL[0:HD, cs], COPYF)
            nc.vector.tensor_copy(out=sumsTc[ck][64:128, :],
                                  in_=ctxL[64:128, cs])
            # full-tile recip (base 0, 128 partitions): rows 0:64 hold
            # garbage whose recips are unused
            nc.vector.reciprocal_approx_fast(out=recfTc[ck],
                                             in_=sumsTc[ck])
            nc.vector.tensor_copy(out=recbT[64:65, cs],
                                  in_=recfTc[ck][64:65, :])
            rb = ps_pool.tile([128, 512], F32, tag="work", name=f"rbt{ck}")
            nc.tensor.matmul(rb[0:64, :], lhsT=ones64[64:65, :],
                             rhs=recbT[64:65, cs], start=True, stop=True)
            nc.vector.tensor_tensor(ctxa[2][0:HD, cs],
                                    ctxu[hL][0:HD, cs],
                                    rb[0:64, :], MULT)
            # out projection for the 4 seq tiles this chunk covers
            for si in range(ck * 4, ck * 4 + 4):
                op = ps_pool.tile([128, 1024], F32, tag="work", name=f"o{si}")
                for c in range(3):
                    nc.tensor.matmul(
                        op[:, 0:512],
                        lhsT=ctxa[c][:, si * 128:(si + 1) * 128],
                        rhs=ow_sb[c][:, 0:512],
                        start=(c == 0), stop=(c == 2))
                    nc.tensor.matmul(
                        op[:, 512:D],
                        lhsT=ctxa[c][:, si * 128:(si + 1) * 128],
                        rhs=ow_sb[c][:, 512:D],
                        start=(c == 0), stop=(c == 2))
                ot = o_pool.tile([128, D], BBF, tag="o", name=f"ot{si}")
                if si % 2 == 0:
                    nc.vector.tensor_copy(out=ot, in_=op[:, 0:D])
                else:
                    nc.scalar.copy(out=ot, in_=op[:, 0:D])
                # sync HWDGE only: SWDGE's trigger->execute latency would
                # stall the final transfer
                nc.sync.dma_start(out=out[si * 128:(si + 1) * 128, :], in_=ot)

    return nc


def _get_nc(finalized=False):
    if "nc" not in _CACHE:
        _CACHE["nc"] = _build_bass()
    nc = _CACHE["nc"]
    if finalized and not nc.is_finalized():
        nc.finalize()
    return nc


def _prep_core_inputs(inputs, core):
    """Host-side shard prep for one core."""
    hs = np.asarray(inputs["hidden_states"], np.float32)
    mask = np.asarray(inputs["attention_mask"])
    q_w = np.asarray(inputs["q_w"], np.float32)
    q_b = np.asarray(inputs["q_b"], np.float32)
    k_w = np.asarray(inputs["k_w"], np.float32)
    k_b = np.asarray(inputs["k_b"], np.float32)
    v_w = np.asarray(inputs["v_w"], np.float32)
    v_b = np.asarray(inputs["v_b"], np.float32)
    out_w = np.asarray(inputs["out_w"], np.float32)

    b, hh = divmod(core, 2)
    hsl = slice(hh * FPC, (hh + 1) * FPC)

    # per-head 65-wide v blocks; even heads (v0..v63, ones) -> ctx rows
    # 0-63 + denom row 64; odd heads (ones, v0..v63) so the on-chip
    # scatter to cols 63..127 puts ones/denom at row 63, v at 64-127.
    wv65 = np.zeros((D, VW), np.float32)
    wvbv = np.zeros((1, VW), np.float32)
    for j in range(HPC):
        fs = hh * FPC + j * HD
        off = j * (HD + 1) + (j % 2)
        wv65[:, off:off + HD] = v_w[fs:fs + HD].T
        wvbv[0, off:off + HD] = v_b[fs:fs + HD]
        wvbv[0, j * (HD + 1) + (0 if j % 2 else HD)] = 1.0

    qkbv = np.empty((128, 6), np.float32)
    qkbv[:, 0:3] = q_b[hsl].reshape(3, 128).T
    qkbv[:, 3:6] = k_b[hsl].reshape(3, 128).T

    dgIv = np.tile(-240.0 * np.eye(128, dtype=np.float32), (1, KT)).astype(E4)

    heads = slice(hh * HPC, (hh + 1) * HPC)
    mT6 = np.ascontiguousarray(mask[0, heads].transpose(0, 2, 1))
    # (1 - m) as fp8 bytes: 1.0 -> 0x38, 0.0 -> 0x00
    mP = np.where(mT6, np.uint8(0), np.uint8(0x38)).reshape(HPC, KT, 128, S)

    return {
        "hsT": np.ascontiguousarray(hs[b].T).astype(BF16),
        "wqT": np.ascontiguousarray(q_w[hsl].T).astype(BF16),
        "wkT": np.ascontiguousarray(k_w[hsl].T).astype(BF16),
        "wvT": wv65.astype(BF16),
        "wvb": wvbv.astype(BF16), "qkb": qkbv,
        "owT": np.ascontiguousarray(
            out_w.T[hsl].reshape(3, 128, D)).astype(BF16),
        "dgI": dgIv, "zrow": np.zeros((64, S), E4), "maskP": mP.view(E4),
    }


def kernel(**inputs):
    global _last_result
    nc = _get_nc(finalized=True)
    in_maps = [_prep_core_inputs(inputs, c) for c in range(NCORES)]
    res = run_bass_kernel_spmd(
        nc, in_maps, core_ids=list(range(NCORES)),
        tmpdir=os.environ.get("KERNEL_TRACE_DIR") or None)
    _last_result = res
    outs = [np.asarray(r["out"], dtype=np.float32) for r in res.results]
    out_b = np.asarray(inputs["out_b"], np.float32)
    full = np.empty((B, S, D), np.float32)
    for b in range(B):
        full[b] = outs[2 * b] + outs[2 * b + 1] + out_b
    return full
